# revision 46
# baseline (speedup 1.0000x reference)
"""Multi-head attention (softmax over the HEADS axis) on 8 trn2 NeuronCores.

Reference math (B=2, S=2048, D=512, H=8, Dk=64):
    q = split_heads(Q @ w_q.T + b_q)          # [B,H,S,Dk]
    scores = q @ k.T / sqrt(Dk)               # [B,H,Sq,Sk]
    probs = softmax(scores, axis=1)           # softmax over H (source quirk!)
    attn = probs @ v                          # [B,H,Sq,Dk]
    out = concat_heads(attn) @ w_o.T + b_o    # [B,S,D]

softmax over H is local to each (b, sq, sk): shard over (batch x query rows),
core c handles batch c//4, query rows (c%4)*512 .. +512, all 8 heads resident.

Steady state is a 3-engine pipeline per kj tile (128 keys), ~5.2us each:
  PE:  scores as 4x 64x64-quadrant matmuls per head-pair (concurrent PE
       sub-arrays, shared-rhs streaming) + attn pairs + interleaved K/V
       projection chunks
  ACT: exp (4 instrs of [128,2,512] from PSUM, scale folded) + PSUM->SBUF
       projection copies
  DVE: head-sum tree at 2x bf16, fused add+reciprocal custom op, normalize
       muls via broadcast-r (stride-0 AP)
  GpSimd stays idle on purpose: it shares the DVE SBUF port; concurrent
  gpsimd streaming makes DVE ops ~3.7x slower (measured).

Layouts (contraction dim always on SBUF partitions):
  qt/kt/vt  [128, 4, S*]  = X.T      (din = chunk*128 + p), bf16
  w*t       [128, 4, 512] = W.T      (din = chunk*128 + p), bf16
  qTs/kTs   [128, 4, S*]  = proj.T   (dout = m*128 + p), bf16
  vs        [128, 16, 512] = v natural (kj on partitions), bf16
  scores    psum [kj=128, 2, 512qi] per head pair -> exp -> softmax over h
  attn      psum [2*64=128 d, 512 qi] per head-pair, accumulated over kj tiles
  out       [qi, 512] natural, fp32
"""

import numpy as np

B, S, D, H, DK = 2, 2048, 512, 8, 64
NCORES = 8
CPB = NCORES // B          # cores per batch
QI = S // CPB              # query rows per core (512)
KJT = 128                  # kj tile (partition dim of scores)
NKJ = S // KJT             # 16 kj tiles
NC_, CH = 128, 4           # partitions, din chunks
SCALE = 1.0 / np.sqrt(DK)  # folded into exp activation


def _chunk(x, dt):
    """[512, F] -> [128, 4, F] with row = chunk*128 + p."""
    f = x.shape[1]
    return np.ascontiguousarray(
        np.ascontiguousarray(x).reshape(CH, NC_, f).transpose(1, 0, 2)
    ).astype(dt)


def _recip_sum_op():
    """Register (once) a fused DVE op: out = recip_approx(Src0 + Src1).

    Folds the last level of the head-sum tree into the reciprocal seed +
    one Newton step (6 ALU stages, fits the 8-slice pipeline; ~0.17% max
    rel err vs the 2-step variant's ~51 ULP).  Saves one full DVE pass
    per kj tile.
    """
    import concourse.dve_ops as dvo
    from concourse.dve_spec import AluOp, Bin, Spec, lower
    from concourse.dve_uop import DveOpSpec

    name = "RECIP_SUM_NR1_ANT"
    for op in dvo.OPS:
        if op.name == name:
            return op

    from concourse.dve_spec import C0, C1, Src0, Src1

    s = Src0 + Src1
    _not = Bin(AluOp.BITWISE_NOT, s, s)
    y0 = _not * C0

    def _ref(in0, in1, c0, c1, c2):
        x = in0.astype(np.float32) + in1.astype(np.float32)
        nx = (~x.view(np.int32)).view(np.float32)
        y0 = nx * c0
        return (y0 * (c1 - x * y0)).astype(np.float32)

    spec = Spec(body=y0 * (C1 - s * y0), reference=_ref)
    row = dvo._CUSTOM_DVE_ROW_BASE + len(dvo.OPS)
    dvo._SUB_OPCODE_FOR_NAME[name] = row
    shas = {}
    for ver in ("v3", "v4"):
        shas[ver] = DveOpSpec(
            name=name, opcode=row, uops=lower(spec, ver=ver), rd1_en=True
        ).sha(ver)
    op = dvo.DveOp(
        name, spec, subdim=False, uops_sha=shas,
        perf_en={"v3": True, "v4": True},
    )
    dvo.OPS.append(op)
    dvo.CUSTOM_DVE_SPECS[name] = spec
    return op


def _build(with_bias):
    from contextlib import ExitStack

    import concourse.bass as bass
    import concourse.mybir as mybir
    import concourse.tile as tile
    from concourse import bacc

    fp32 = mybir.dt.float32
    bf16 = mybir.dt.bfloat16

    nc = bacc.Bacc(
        "TRN2",
        target_bir_lowering=False,
        debug=False,
        enable_asserts=False,
        num_devices=NCORES,
    )

    def din(name, shape):
        return nc.dram_tensor(name, shape, bf16, kind="ExternalInput").ap()

    qt_d = din("qt", [NC_, CH, QI])
    kt_d = din("kt", [NC_, CH, S])
    vt_d = din("vt", [NC_, CH, S])
    w_d = {n: din(n, [NC_, CH, D]) for n in ("wqt", "wkt", "wvt", "wot")}
    if with_bias:
        b_d = {n: din(n, [1, D]) for n in ("bq", "bk", "bv", "bo")}
    out_d = nc.dram_tensor("out", [QI, D], fp32, kind="ExternalOutput").ap()

    with tile.TileContext(nc) as tc, ExitStack() as ctx:
        wpool = ctx.enter_context(tc.tile_pool(name="wts", bufs=4))
        raw = ctx.enter_context(tc.tile_pool(name="raw", bufs=9))
        acts = ctx.enter_context(tc.tile_pool(name="acts", bufs=1))
        sm = ctx.enter_context(tc.tile_pool(name="sm", bufs=4))
        pp = ctx.enter_context(tc.tile_pool(name="pp", bufs=8))
        ps = ctx.enter_context(tc.tile_pool(name="ps", bufs=2, space="PSUM"))
        psa = ctx.enter_context(tc.tile_pool(name="psa", bufs=4, space="PSUM"))

        qTs = acts.tile([NC_, CH, QI], bf16, tag="qTs")
        kTs = acts.tile([NC_, CH, S], bf16, tag="kTs")
        vs = acts.tile([NC_, NKJ, D], bf16, tag="vs")
        attnT = acts.tile([NC_, CH, QI], bf16, tag="attnT")
        outsb = acts.tile([NC_, CH, D], fp32, tag="outsb")

        if with_bias:
            ones = acts.tile([1, D], bf16, tag="ones")
            nc.vector.memset(ones, 1.0)
            brow = {}
            for n in ("bq", "bk", "bv", "bo"):
                brow[n] = acts.tile([1, D], bf16, tag=n, name=n)
                nc.sync.dma_start(out=brow[n], in_=b_d[n])

        wsb = {}
        for n in ("wqt", "wkt", "wvt", "wot"):
            wsb[n] = wpool.tile([NC_, CH, D], bf16, tag="w", name=n)

        def bias_mm(pt_ap, bname, col_slice):
            """rank-1 bias init: psum = bias-row (x) ones-row (or flipped)."""
            if col_slice is not None:  # bias along partitions
                lhsT = brow[bname][:, col_slice]
                rhs = ones[:, : pt_ap.shape[-1]]
            else:  # bias along free dim
                lhsT = ones[:, :128]
                rhs = brow[bname]
            nc.tensor.matmul(pt_ap, lhsT=lhsT, rhs=rhs, start=True, stop=False)

        # warm the exp table-set while input DMAs are in flight
        scr = acts.tile([1, 8], fp32, tag="scr")
        nc.vector.memset(scr, 1.0)
        nc.scalar.activation(
            scr, scr, mybir.ActivationFunctionType.Exp, scale=1.0
        )

        # ---------------- input DMAs ----------------
        # priority order: everything Q proj needs, then K kc-chunks (scores
        # tile 0 starts after ~1/4 of the K load), then V chunks, w_o last.
        qraw = raw.tile([NC_, CH, QI], bf16, tag="raw")
        kraw = [raw.tile([NC_, S], bf16, tag="raw", name=f"kraw{c}") for c in range(CH)]
        vraw = [raw.tile([NC_, S], bf16, tag="raw", name=f"vraw{c}") for c in range(CH)]
        for c in range(CH):
            nc.sync.dma_start(out=wsb["wqt"][:, c, :], in_=w_d["wqt"][:, c, :])
            nc.sync.dma_start(out=qraw[:, c, :], in_=qt_d[:, c, :])
        for c in range(CH):
            nc.sync.dma_start(out=wsb["wkt"][:, c, :], in_=w_d["wkt"][:, c, :])
            nc.sync.dma_start(out=kraw[c][:, 0:512], in_=kt_d[:, c, 0:512])
        nc.sync.dma_start(out=wsb["wvt"], in_=w_d["wvt"])
        for c in range(CH):
            nc.sync.dma_start(out=vraw[c][:, 0:512], in_=vt_d[:, c, 0:512])
        for kc in range(1, 4):
            for c in range(CH):
                nc.sync.dma_start(
                    out=kraw[c][:, kc * 512 : (kc + 1) * 512],
                    in_=kt_d[:, c, kc * 512 : (kc + 1) * 512],
                )
            for c in range(CH):
                nc.sync.dma_start(
                    out=vraw[c][:, kc * 512 : (kc + 1) * 512],
                    in_=vt_d[:, c, kc * 512 : (kc + 1) * 512],
                )

        # ---------------- projection emitters ----------------
        # Q proj: qT[dout, qi]; two m per psum tile, one [128,1024] copy
        def emit_q_proj(mp):
            pt = ps.tile([NC_, 2, 512], fp32, tag="ps", name=f"qp{mp}")
            for j in range(2):
                m = 2 * mp + j
                if with_bias:
                    bias_mm(pt[:, j, :QI], "bq", slice(m * 128, (m + 1) * 128))
                for c in range(CH):
                    nc.tensor.matmul(
                        pt[:, j, :QI],
                        lhsT=wsb["wqt"][:, c, m * 128 : (m + 1) * 128],
                        rhs=qraw[:, c, :],
                        start=(c == 0 and not with_bias),
                        stop=(c == CH - 1),
                    )
            nc.vector.tensor_copy(qTs[:, 2 * mp : 2 * mp + 2, :], pt[:, :, :QI])

        copyq = []

        # K proj, one kc chunk (kj range kc*512..+512), one m-pair
        def emit_k_proj(kc, mp, eng=None):
            pt = ps.tile([NC_, 2, 512], fp32, tag="ps", name=f"kp{kc}_{mp}")
            for j in range(2):
                m = 2 * mp + j
                if with_bias:
                    bias_mm(pt[:, j, :], "bk", slice(m * 128, (m + 1) * 128))
                for c in range(CH):
                    nc.tensor.matmul(
                        pt[:, j, :],
                        lhsT=wsb["wkt"][:, c, m * 128 : (m + 1) * 128],
                        rhs=kraw[c][:, kc * 512 : (kc + 1) * 512],
                        start=(c == 0 and not with_bias),
                        stop=(c == CH - 1),
                    )
            dst = kTs[:, 2 * mp : 2 * mp + 2, kc * 512 : (kc + 1) * 512]
            if eng is not None:
                eng.tensor_copy(dst, pt[:, :, :])
            else:
                copyq.append((dst, pt))

        # V proj for kj tiles (t, t+1): v natural [kj, dout]
        def emit_v_proj(t, eng=None):
            pt = ps.tile([NC_, 2, 512], fp32, tag="ps", name=f"vp{t}")
            for j in range(2):
                tt = t + j
                if with_bias:
                    bias_mm(pt[:, j, :], "bv", None)
                for c in range(CH):
                    nc.tensor.matmul(
                        pt[:, j, :],
                        lhsT=vraw[c][:, tt * 128 : (tt + 1) * 128],
                        rhs=wsb["wvt"][:, c, :],
                        start=(c == 0 and not with_bias),
                        stop=(c == CH - 1),
                    )
            if eng is not None:
                eng.tensor_copy(vs[:, t : t + 2, :], pt)
            else:
                copyq.append((vs[:, t : t + 2, :], pt))

        # ---------------- attention ----------------
        # attn psum: tile dc holds heads 2dc (p 0..63), 2dc+1 (p 64..127)
        at = [psa.tile([NC_, 512], fp32, tag="attn", name=f"at{i}") for i in range(4)]

        def emit_attn(td, prs):
            for h in range(H):
                po = (h % 2) * 64
                nc.tensor.matmul(
                    at[h // 2][po : po + 64, :QI],
                    lhsT=vs[:, td, h * 64 : (h + 1) * 64],
                    rhs=prs[h // 4][:, h % 4, :],
                    start=(td == 0),
                    stop=(td == NKJ - 1),
                )
                if td == NKJ - 1 and h % 2 == 1:
                    dc = h // 2
                    if dc % 2 == 0:
                        nc.vector.tensor_copy(attnT[:, dc, :], at[dc][:, :QI])
                    else:
                        nc.scalar.copy(attnT[:, dc, :], at[dc][:, :QI])

        from concourse.dve_ops import RECIP_APPROX_FAST_CONSTS as _RC

        _RS = _recip_sum_op()

        # prologue: only what scores tile 0 heads 0-3 need; the rest of
        # Q/K0 proj is emitted mid-tile-0 so the first exp fires earlier
        emit_q_proj(0)
        emit_k_proj(0, 0, eng=nc.vector)
        nc.sync.dma_start(out=wsb["wot"], in_=w_d["wot"])

        LAG = 3
        pending = []
        for t in range(NKJ):
            exp_t = sm.tile([NC_, H, QI], bf16, tag="exp", bufs=4)
            for m in range(4):
                if t == 0 and m == 2:
                    emit_q_proj(1)
                    emit_k_proj(0, 1, eng=nc.vector)
                spt = ps.tile([NC_, 2, 512], fp32, tag="ps")
                # 4 quadrant matmuls (64 contraction x 64 out-partitions):
                # disjoint (row_grp, col_grp) -> PE sub-array concurrency
                for j in range(2):       # head parity (dk rows 0:64 / 64:128)
                    po = j * 64
                    for kh in range(2):  # kj half (out partitions 0:64 / 64:128)
                        ko = kh * 64
                        nc.tensor.matmul(
                            spt[ko : ko + 64, j, :QI],
                            lhsT=kTs[
                                po : po + 64, m, t * 128 + ko : t * 128 + ko + 64
                            ],
                            rhs=qTs[po : po + 64, m, :],
                            start=True,
                            stop=True,
                        )
                nc.scalar.activation(
                    exp_t[:, 2 * m : 2 * m + 2, :],
                    spt[:, :, :],
                    mybir.ActivationFunctionType.Exp,
                    scale=SCALE,
                )

            # projections run ~2-4 tiles ahead of their consumers, emitted
            # after this tile's scores so they never delay the softmax chain
            if t == 0:
                emit_v_proj(0, eng=nc.vector)
                emit_v_proj(2, eng=nc.vector)
            if t % 4 in (2, 3) and t // 4 + 1 < 4:
                emit_k_proj(t // 4 + 1, t % 4 - 2)
            if t % 2 == 0 and t + 4 < NKJ:
                emit_v_proj(t + 4)

            # head-sum tree, all on DVE at 2x (gpsimd steals the shared SBUF
            # port and makes concurrent DVE ops ~3.7x slower -- keep it idle)
            s4 = sm.tile([NC_, 4, QI], bf16, tag="s4", bufs=2)
            nc.vector.tensor_add(s4, exp_t[:, 0:4, :], exp_t[:, 4:8, :])
            s2 = sm.tile([NC_, 2, QI], bf16, tag="s2", bufs=2)
            nc.vector.tensor_add(s2, s4[:, 0:2, :], s4[:, 2:4, :])
            # fused final-add + fast reciprocal (bf16 out; the bit-trick
            # runs on the fp32 sum computed inside the DVE pipeline)
            r = sm.tile([NC_, QI], bf16, tag="r", bufs=3)
            nc.vector._custom_dve(
                _RS,
                out=r,
                in0=s2[:, 0, :],
                in1=s2[:, 1, :],
                s0=_RC["s0"],
                s1=_RC["s1"],
                imm2=0.0,
            )

            # normalize: broadcast-r (stride-0 middle dim) keeps DVE at 2x with
            # one instr per 4-head group
            prs = []
            rb4 = r.unsqueeze(1).broadcast_to((NC_, 4, QI))
            for g in range(2):
                pr = pp.tile([NC_, 4, QI], bf16, tag="probs")
                nc.vector.tensor_mul(pr, exp_t[:, 4 * g : 4 * g + 4, :], rb4)
                prs.append(pr)

            # proj copies slot into Scalar idle time (deprioritized so
            # the scheduler prefers the exp chain that gates DVE)
            with tc.high_priority(offset=-64):
                while copyq:
                    dst, pt = copyq.pop(0)
                    nc.scalar.copy(dst, pt)

            # attn matmuls run LAG tiles behind (probs already ready -> PE
            # never stalls mid-stream on the softmax chain); the lag tapers
            # off over the last tiles so the drain after the loop is short
            pending.append((t, prs))
            lag_now = min(LAG, NKJ - 1 - t)
            while len(pending) > lag_now:
                emit_attn(*pending.pop(0))

        while copyq:
            dst, pt = copyq.pop(0)
            nc.scalar.copy(dst, pt)
        for td, prs in pending:
            emit_attn(td, prs)

        # ---------------- output projection ----------------
        for m in range(4):
            ot = psa.tile([NC_, 512], fp32, tag="attn")
            if with_bias:
                bias_mm(ot, "bo", None)
            for c in range(CH):
                nc.tensor.matmul(
                    ot,
                    lhsT=attnT[:, c, m * 128 : (m + 1) * 128],
                    rhs=wsb["wot"][:, c, :],
                    start=(c == 0 and not with_bias),
                    stop=(c == CH - 1),
                )
            if m % 2 == 0:
                nc.scalar.copy(outsb[:, m, :], ot)
            else:
                nc.vector.tensor_copy(outsb[:, m, :], ot)
            nc.sync.dma_start(
                out=out_d.rearrange("(m p) o -> p m o", p=NC_)[:, m, :],
                in_=outsb[:, m, :],
            )

    nc.compile()
    return nc


_CACHE = {}


def kernel(Q, K, V, w_q, b_q, w_k, b_k, w_v, b_v, w_o, b_o, _trace=False):
    import ml_dtypes
    from concourse import bass_utils

    bf = ml_dtypes.bfloat16
    Q = np.asarray(Q, np.float32)
    K = np.asarray(K, np.float32)
    V = np.asarray(V, np.float32)
    with_bias = any(
        np.any(np.asarray(b) != 0) for b in (b_q, b_k, b_v, b_o)
    )

    if ("nc", with_bias) not in _CACHE:
        _CACHE[("nc", with_bias)] = _build(with_bias)
    nc = _CACHE[("nc", with_bias)]

    wmaps = {
        "wqt": _chunk(np.asarray(w_q, np.float32).T, bf),
        "wkt": _chunk(np.asarray(w_k, np.float32).T, bf),
        "wvt": _chunk(np.asarray(w_v, np.float32).T, bf),
        "wot": _chunk(np.asarray(w_o, np.float32).T, bf),
    }
    if with_bias:
        for n, b in (("bq", b_q), ("bk", b_k), ("bv", b_v), ("bo", b_o)):
            wmaps[n] = np.ascontiguousarray(
                np.asarray(b, np.float32).reshape(1, D)
            ).astype(bf)

    in_maps = []
    for c in range(NCORES):
        b = c // CPB
        s0 = (c % CPB) * QI
        in_maps.append(
            dict(
                wmaps,
                qt=_chunk(Q[b, s0 : s0 + QI, :].T, bf),
                kt=_chunk(K[b].T, bf),
                vt=_chunk(V[b].T, bf),
            )
        )

    res = bass_utils.run_bass_kernel_spmd(
        nc, in_maps, core_ids=list(range(NCORES)), trace=_trace
    )

    out = np.empty((B, S, D), np.float32)
    for c in range(NCORES):
        b = c // CPB
        s0 = (c % CPB) * QI
        out[b, s0 : s0 + QI, :] = res.results[c]["out"]
    if _trace:
        kernel._last_results = res
    return out


# revision 47
# speedup vs baseline: 1.0198x; 1.0198x over previous
"""Multi-head attention (softmax over the HEADS axis) on 8 trn2 NeuronCores.

Reference math (B=2, S=2048, D=512, H=8, Dk=64):
    q = split_heads(Q @ w_q.T + b_q)          # [B,H,S,Dk]
    scores = q @ k.T / sqrt(Dk)               # [B,H,Sq,Sk]
    probs = softmax(scores, axis=1)           # softmax over H (source quirk!)
    attn = probs @ v                          # [B,H,Sq,Dk]
    out = concat_heads(attn) @ w_o.T + b_o    # [B,S,D]

softmax over H is local to each (b, sq, sk): shard over (batch x query rows),
core c handles batch c//4, query rows (c%4)*512 .. +512, all 8 heads resident.

Steady state is a 3-engine pipeline per kj tile (128 keys), ~5.2us each:
  PE:  scores as 4x 64x64-quadrant matmuls per head-pair (concurrent PE
       sub-arrays, shared-rhs streaming) + attn pairs + interleaved K/V
       projection chunks
  ACT: exp (4 instrs of [128,2,512] from PSUM, scale folded) + PSUM->SBUF
       projection copies
  DVE: head-sum tree at 2x bf16, fused add+reciprocal custom op, normalize
       muls via broadcast-r (stride-0 AP)
  GpSimd stays idle on purpose: it shares the DVE SBUF port; concurrent
  gpsimd streaming makes DVE ops ~3.7x slower (measured).

Layouts (contraction dim always on SBUF partitions):
  qt/kt/vt  [128, 4, S*]  = X.T      (din = chunk*128 + p), bf16
  w*t       [128, 4, 512] = W.T      (din = chunk*128 + p), bf16
  qTs/kTs   [128, 4, S*]  = proj.T   (dout = m*128 + p), bf16
  vs        [128, 16, 512] = v natural (kj on partitions), bf16
  scores    psum [kj=128, 2, 512qi] per head pair -> exp -> softmax over h
  attn      psum [2*64=128 d, 512 qi] per head-pair, accumulated over kj tiles
  out       [qi, 512] natural, fp32
"""

import numpy as np

B, S, D, H, DK = 2, 2048, 512, 8, 64
NCORES = 8
CPB = NCORES // B          # cores per batch
QI = S // CPB              # query rows per core (512)
KJT = 128                  # kj tile (partition dim of scores)
NKJ = S // KJT             # 16 kj tiles
NC_, CH = 128, 4           # partitions, din chunks
SCALE = 1.0 / np.sqrt(DK)  # folded into exp activation


def _chunk(x, dt):
    """[512, F] -> [128, 4, F] with row = chunk*128 + p."""
    f = x.shape[1]
    return np.ascontiguousarray(
        np.ascontiguousarray(x).reshape(CH, NC_, f).transpose(1, 0, 2)
    ).astype(dt)


def _recip_sum_op():
    """Register (once) a fused DVE op: out = recip_approx(Src0 + Src1).

    Folds the last level of the head-sum tree into the reciprocal seed +
    one Newton step (6 ALU stages, fits the 8-slice pipeline; ~0.17% max
    rel err vs the 2-step variant's ~51 ULP).  Saves one full DVE pass
    per kj tile.
    """
    import concourse.dve_ops as dvo
    from concourse.dve_spec import AluOp, Bin, Spec, lower
    from concourse.dve_uop import DveOpSpec

    name = "RECIP_SUM_NR1_ANT"
    for op in dvo.OPS:
        if op.name == name:
            return op

    from concourse.dve_spec import C0, C1, Src0, Src1

    s = Src0 + Src1
    _not = Bin(AluOp.BITWISE_NOT, s, s)
    y0 = _not * C0

    def _ref(in0, in1, c0, c1, c2):
        x = in0.astype(np.float32) + in1.astype(np.float32)
        nx = (~x.view(np.int32)).view(np.float32)
        y0 = nx * c0
        return (y0 * (c1 - x * y0)).astype(np.float32)

    spec = Spec(body=y0 * (C1 - s * y0), reference=_ref)
    row = dvo._CUSTOM_DVE_ROW_BASE + len(dvo.OPS)
    dvo._SUB_OPCODE_FOR_NAME[name] = row
    shas = {}
    for ver in ("v3", "v4"):
        shas[ver] = DveOpSpec(
            name=name, opcode=row, uops=lower(spec, ver=ver), rd1_en=True
        ).sha(ver)
    op = dvo.DveOp(
        name, spec, subdim=False, uops_sha=shas,
        perf_en={"v3": True, "v4": True},
    )
    dvo.OPS.append(op)
    dvo.CUSTOM_DVE_SPECS[name] = spec
    return op


def _build(with_bias):
    from contextlib import ExitStack

    import concourse.bass as bass
    import concourse.mybir as mybir
    import concourse.tile as tile
    from concourse import bacc

    fp32 = mybir.dt.float32
    bf16 = mybir.dt.bfloat16

    nc = bacc.Bacc(
        "TRN2",
        target_bir_lowering=False,
        debug=False,
        enable_asserts=False,
        num_devices=NCORES,
    )

    def din(name, shape):
        return nc.dram_tensor(name, shape, bf16, kind="ExternalInput").ap()

    qt_d = din("qt", [NC_, CH, QI])
    kt_d = din("kt", [NC_, CH, S])
    vt_d = din("vt", [NC_, CH, S])
    w_d = {n: din(n, [NC_, CH, D]) for n in ("wqt", "wkt", "wvt", "wot")}
    if with_bias:
        b_d = {n: din(n, [1, D]) for n in ("bq", "bk", "bv", "bo")}
    out_d = nc.dram_tensor("out", [QI, D], fp32, kind="ExternalOutput").ap()

    with tile.TileContext(nc) as tc, ExitStack() as ctx:
        wpool = ctx.enter_context(tc.tile_pool(name="wts", bufs=4))
        raw = ctx.enter_context(tc.tile_pool(name="raw", bufs=9))
        acts = ctx.enter_context(tc.tile_pool(name="acts", bufs=1))
        sm = ctx.enter_context(tc.tile_pool(name="sm", bufs=4))
        pp = ctx.enter_context(tc.tile_pool(name="pp", bufs=8))
        ps = ctx.enter_context(tc.tile_pool(name="ps", bufs=2, space="PSUM"))
        psa = ctx.enter_context(tc.tile_pool(name="psa", bufs=4, space="PSUM"))

        qTs = acts.tile([NC_, CH, QI], bf16, tag="qTs")
        kTs = acts.tile([NC_, CH, S], bf16, tag="kTs")
        vs = acts.tile([NC_, NKJ, D], bf16, tag="vs")
        attnT = acts.tile([NC_, CH, QI], bf16, tag="attnT")
        outsb = acts.tile([NC_, CH, D], fp32, tag="outsb")

        if with_bias:
            ones = acts.tile([1, D], bf16, tag="ones")
            nc.vector.memset(ones, 1.0)
            brow = {}
            for n in ("bq", "bk", "bv", "bo"):
                brow[n] = acts.tile([1, D], bf16, tag=n, name=n)
                nc.sync.dma_start(out=brow[n], in_=b_d[n])

        wsb = {}
        for n in ("wqt", "wkt", "wvt", "wot"):
            wsb[n] = wpool.tile([NC_, CH, D], bf16, tag="w", name=n)

        def bias_mm(pt_ap, bname, col_slice):
            """rank-1 bias init: psum = bias-row (x) ones-row (or flipped)."""
            if col_slice is not None:  # bias along partitions
                lhsT = brow[bname][:, col_slice]
                rhs = ones[:, : pt_ap.shape[-1]]
            else:  # bias along free dim
                lhsT = ones[:, :128]
                rhs = brow[bname]
            nc.tensor.matmul(pt_ap, lhsT=lhsT, rhs=rhs, start=True, stop=False)

        # warm the exp table-set while input DMAs are in flight
        scr = acts.tile([1, 8], fp32, tag="scr")
        nc.vector.memset(scr, 1.0)
        nc.scalar.activation(
            scr, scr, mybir.ActivationFunctionType.Exp, scale=1.0
        )

        # ---------------- input DMAs ----------------
        # priority order: everything Q proj needs, then K kc-chunks (scores
        # tile 0 starts after ~1/4 of the K load), then V chunks, w_o last.
        qraw = raw.tile([NC_, CH, QI], bf16, tag="raw")
        kraw = [raw.tile([NC_, S], bf16, tag="raw", name=f"kraw{c}") for c in range(CH)]
        vraw = [raw.tile([NC_, S], bf16, tag="raw", name=f"vraw{c}") for c in range(CH)]
        for c in range(CH):
            nc.sync.dma_start(out=wsb["wqt"][:, c, :], in_=w_d["wqt"][:, c, :])
            nc.sync.dma_start(out=qraw[:, c, :], in_=qt_d[:, c, :])
        for c in range(CH):
            nc.sync.dma_start(out=wsb["wkt"][:, c, :], in_=w_d["wkt"][:, c, :])
            nc.sync.dma_start(out=kraw[c][:, 0:512], in_=kt_d[:, c, 0:512])
        nc.sync.dma_start(out=wsb["wvt"], in_=w_d["wvt"])
        for c in range(CH):
            nc.sync.dma_start(out=vraw[c][:, 0:512], in_=vt_d[:, c, 0:512])
        for kc in range(1, 4):
            for c in range(CH):
                nc.sync.dma_start(
                    out=kraw[c][:, kc * 512 : (kc + 1) * 512],
                    in_=kt_d[:, c, kc * 512 : (kc + 1) * 512],
                )
            for c in range(CH):
                nc.sync.dma_start(
                    out=vraw[c][:, kc * 512 : (kc + 1) * 512],
                    in_=vt_d[:, c, kc * 512 : (kc + 1) * 512],
                )

        # ---------------- projection emitters ----------------
        # Q proj: qT[dout, qi]; two m per psum tile, one [128,1024] copy
        def emit_q_proj(mp):
            pt = ps.tile([NC_, 2, 512], fp32, tag="ps", name=f"qp{mp}")
            for j in range(2):
                m = 2 * mp + j
                if with_bias:
                    bias_mm(pt[:, j, :QI], "bq", slice(m * 128, (m + 1) * 128))
                for c in range(CH):
                    nc.tensor.matmul(
                        pt[:, j, :QI],
                        lhsT=wsb["wqt"][:, c, m * 128 : (m + 1) * 128],
                        rhs=qraw[:, c, :],
                        start=(c == 0 and not with_bias),
                        stop=(c == CH - 1),
                    )
            nc.vector.tensor_copy(qTs[:, 2 * mp : 2 * mp + 2, :], pt[:, :, :QI])

        copyq = []

        # K proj, one kc chunk (kj range kc*512..+512), one m-pair
        def emit_k_proj(kc, mp, eng=None):
            pt = ps.tile([NC_, 2, 512], fp32, tag="ps", name=f"kp{kc}_{mp}")
            for j in range(2):
                m = 2 * mp + j
                if with_bias:
                    bias_mm(pt[:, j, :], "bk", slice(m * 128, (m + 1) * 128))
                for c in range(CH):
                    nc.tensor.matmul(
                        pt[:, j, :],
                        lhsT=wsb["wkt"][:, c, m * 128 : (m + 1) * 128],
                        rhs=kraw[c][:, kc * 512 : (kc + 1) * 512],
                        start=(c == 0 and not with_bias),
                        stop=(c == CH - 1),
                    )
            dst = kTs[:, 2 * mp : 2 * mp + 2, kc * 512 : (kc + 1) * 512]
            if eng is not None:
                eng.tensor_copy(dst, pt[:, :, :])
            else:
                copyq.append((dst, pt))

        # V proj for kj tiles (t, t+1): v natural [kj, dout]
        def emit_v_proj(t, eng=None):
            pt = ps.tile([NC_, 2, 512], fp32, tag="ps", name=f"vp{t}")
            for j in range(2):
                tt = t + j
                if with_bias:
                    bias_mm(pt[:, j, :], "bv", None)
                for c in range(CH):
                    nc.tensor.matmul(
                        pt[:, j, :],
                        lhsT=vraw[c][:, tt * 128 : (tt + 1) * 128],
                        rhs=wsb["wvt"][:, c, :],
                        start=(c == 0 and not with_bias),
                        stop=(c == CH - 1),
                    )
            if eng is not None:
                eng.tensor_copy(vs[:, t : t + 2, :], pt)
            else:
                copyq.append((vs[:, t : t + 2, :], pt))

        # ---------------- attention ----------------
        # attn psum: tile dc holds heads 2dc (p 0..63), 2dc+1 (p 64..127)
        at = [psa.tile([NC_, 512], fp32, tag="attn", name=f"at{i}") for i in range(4)]

        def emit_attn(td, prs):
            for h in range(H):
                po = (h % 2) * 64
                nc.tensor.matmul(
                    at[h // 2][po : po + 64, :QI],
                    lhsT=vs[:, td, h * 64 : (h + 1) * 64],
                    rhs=prs[h // 4][:, h % 4, :],
                    start=(td == 0),
                    stop=(td == NKJ - 1),
                )
                if td == NKJ - 1 and h % 2 == 1:
                    dc = h // 2
                    if dc % 2 == 0:
                        nc.vector.tensor_copy(attnT[:, dc, :], at[dc][:, :QI])
                    else:
                        nc.scalar.copy(attnT[:, dc, :], at[dc][:, :QI])

        from concourse.dve_ops import RECIP_APPROX_FAST_CONSTS as _RC

        _RS = _recip_sum_op()

        # prologue: only what scores tile 0 heads 0-3 need; the rest of
        # Q/K0 proj is emitted mid-tile-0 so the first exp fires earlier
        emit_q_proj(0)
        emit_k_proj(0, 0, eng=nc.vector)
        nc.sync.dma_start(out=wsb["wot"], in_=w_d["wot"])

        LAG = 2
        pending = []
        for t in range(NKJ):
            exp_t = sm.tile([NC_, H, QI], bf16, tag="exp", bufs=4)
            for m in range(4):
                if t == 0 and m == 2:
                    emit_q_proj(1)
                    emit_k_proj(0, 1, eng=nc.vector)
                spt = ps.tile([NC_, 2, 512], fp32, tag="ps")
                # 4 quadrant matmuls (64 contraction x 64 out-partitions):
                # disjoint (row_grp, col_grp) -> PE sub-array concurrency
                for j in range(2):       # head parity (dk rows 0:64 / 64:128)
                    po = j * 64
                    for kh in range(2):  # kj half (out partitions 0:64 / 64:128)
                        ko = kh * 64
                        nc.tensor.matmul(
                            spt[ko : ko + 64, j, :QI],
                            lhsT=kTs[
                                po : po + 64, m, t * 128 + ko : t * 128 + ko + 64
                            ],
                            rhs=qTs[po : po + 64, m, :],
                            start=True,
                            stop=True,
                        )
                nc.scalar.activation(
                    exp_t[:, 2 * m : 2 * m + 2, :],
                    spt[:, :, :],
                    mybir.ActivationFunctionType.Exp,
                    scale=SCALE,
                )

            # projections run ~2-4 tiles ahead of their consumers, emitted
            # after this tile's scores so they never delay the softmax chain
            if t == 0:
                emit_v_proj(0, eng=nc.vector)
                emit_v_proj(2, eng=nc.vector)
            if t % 4 in (2, 3) and t // 4 + 1 < 4:
                emit_k_proj(t // 4 + 1, t % 4 - 2)
            if t % 2 == 0 and t + 4 < NKJ:
                emit_v_proj(t + 4)

            # head-sum tree, all on DVE at 2x (gpsimd steals the shared SBUF
            # port and makes concurrent DVE ops ~3.7x slower -- keep it idle)
            s4 = sm.tile([NC_, 4, QI], bf16, tag="s4", bufs=2)
            nc.vector.tensor_add(s4, exp_t[:, 0:4, :], exp_t[:, 4:8, :])
            s2 = sm.tile([NC_, 2, QI], bf16, tag="s2", bufs=2)
            nc.vector.tensor_add(s2, s4[:, 0:2, :], s4[:, 2:4, :])
            # fused final-add + fast reciprocal (bf16 out; the bit-trick
            # runs on the fp32 sum computed inside the DVE pipeline)
            r = sm.tile([NC_, QI], bf16, tag="r", bufs=3)
            nc.vector._custom_dve(
                _RS,
                out=r,
                in0=s2[:, 0, :],
                in1=s2[:, 1, :],
                s0=_RC["s0"],
                s1=_RC["s1"],
                imm2=0.0,
            )

            # normalize: broadcast-r (stride-0 middle dim) keeps DVE at 2x with
            # one instr per 4-head group
            prs = []
            rb4 = r.unsqueeze(1).broadcast_to((NC_, 4, QI))
            for g in range(2):
                pr = pp.tile([NC_, 4, QI], bf16, tag="probs")
                nc.vector.tensor_mul(pr, exp_t[:, 4 * g : 4 * g + 4, :], rb4)
                prs.append(pr)

            # proj copies slot into Scalar idle time (deprioritized so
            # the scheduler prefers the exp chain that gates DVE)
            with tc.high_priority(offset=-64):
                while copyq:
                    dst, pt = copyq.pop(0)
                    nc.scalar.copy(dst, pt)

            # attn matmuls run LAG tiles behind (probs already ready -> PE
            # never stalls mid-stream on the softmax chain); the lag tapers
            # off over the last tiles so the drain after the loop is short
            pending.append((t, prs))
            lag_now = min(LAG, NKJ - 1 - t)
            while len(pending) > lag_now:
                emit_attn(*pending.pop(0))

        while copyq:
            dst, pt = copyq.pop(0)
            nc.scalar.copy(dst, pt)
        for td, prs in pending:
            emit_attn(td, prs)

        # ---------------- output projection ----------------
        for m in range(4):
            ot = psa.tile([NC_, 512], fp32, tag="attn")
            if with_bias:
                bias_mm(ot, "bo", None)
            for c in range(CH):
                nc.tensor.matmul(
                    ot,
                    lhsT=attnT[:, c, m * 128 : (m + 1) * 128],
                    rhs=wsb["wot"][:, c, :],
                    start=(c == 0 and not with_bias),
                    stop=(c == CH - 1),
                )
            if m % 2 == 0:
                nc.scalar.copy(outsb[:, m, :], ot)
            else:
                nc.vector.tensor_copy(outsb[:, m, :], ot)
            nc.sync.dma_start(
                out=out_d.rearrange("(m p) o -> p m o", p=NC_)[:, m, :],
                in_=outsb[:, m, :],
            )

    nc.compile()
    return nc


_CACHE = {}


def kernel(Q, K, V, w_q, b_q, w_k, b_k, w_v, b_v, w_o, b_o, _trace=False):
    import ml_dtypes
    from concourse import bass_utils

    bf = ml_dtypes.bfloat16
    Q = np.asarray(Q, np.float32)
    K = np.asarray(K, np.float32)
    V = np.asarray(V, np.float32)
    with_bias = any(
        np.any(np.asarray(b) != 0) for b in (b_q, b_k, b_v, b_o)
    )

    if ("nc", with_bias) not in _CACHE:
        _CACHE[("nc", with_bias)] = _build(with_bias)
    nc = _CACHE[("nc", with_bias)]

    wmaps = {
        "wqt": _chunk(np.asarray(w_q, np.float32).T, bf),
        "wkt": _chunk(np.asarray(w_k, np.float32).T, bf),
        "wvt": _chunk(np.asarray(w_v, np.float32).T, bf),
        "wot": _chunk(np.asarray(w_o, np.float32).T, bf),
    }
    if with_bias:
        for n, b in (("bq", b_q), ("bk", b_k), ("bv", b_v), ("bo", b_o)):
            wmaps[n] = np.ascontiguousarray(
                np.asarray(b, np.float32).reshape(1, D)
            ).astype(bf)

    in_maps = []
    for c in range(NCORES):
        b = c // CPB
        s0 = (c % CPB) * QI
        in_maps.append(
            dict(
                wmaps,
                qt=_chunk(Q[b, s0 : s0 + QI, :].T, bf),
                kt=_chunk(K[b].T, bf),
                vt=_chunk(V[b].T, bf),
            )
        )

    res = bass_utils.run_bass_kernel_spmd(
        nc, in_maps, core_ids=list(range(NCORES)), trace=_trace
    )

    out = np.empty((B, S, D), np.float32)
    for c in range(NCORES):
        b = c // CPB
        s0 = (c % CPB) * QI
        out[b, s0 : s0 + QI, :] = res.results[c]["out"]
    if _trace:
        kernel._last_results = res
    return out
